# revision 17
# baseline (speedup 1.0000x reference)
"""Trainium2 Bass kernel for AngularSymmetryMod (ANI-style angular symmetry functions).

Math: out[b,i,l] = sum_{j,k} (1+lam*cos(theta-theta_t))^zeta * exp(-ita*((R_ij+R_ik)/2-Rs)^2)
                            * f_ij*f_ik * 2^(1-zeta)
over a 40-point parameter grid l=(lam in {+-1}, 5 Rs values, 4 theta_t values), zeta=4.

Key optimizations:
 1. theta_t = {0.0, 1.57, 3.14, 4.71} are (to 8e-4) the exact quadrants {0, pi/2, pi, 3pi/2},
    so cos(theta-theta_t) = {c, s, -c, -s} and the angular factor collapses to FOUR distinct
    fields: (1+-c)^4, (1+-s)^4 — each two chained Square activations on the ScalarEngine.
    (Validated: 2.2e-4 rel err vs the f32 reference.)
 2. Each of the 40 outputs is S[r, m] (5 radials x 4 angulars = 20 reductions); the 40 outputs
    are a column remap handled by the output DMA.
 3. The (j,k) summand is symmetric, so only the 528 pairs j<=k are computed (host gathers the
    packed pair layout; off-diagonal weight 2 is folded into the cutoff product on-chip).
 4. sin/cos need exact-range reduction (theta spans +-2.3e6): theta/2pi - round(theta/2pi) via
    the DVE's round-to-nearest f32->int32 convert, then the Sin table on [-pi, pi].

Sharding: data-parallel over batch (16 molecules -> 2 per core on 8 cores). No collectives.
Layout per core: 128 partitions = (b_loc:2, i:32, half:2), free = 264 packed (j,k) pairs
(248 off-diagonal + 16 diagonal per half).
"""

import sys
import numpy as np

sys.path.insert(0, "/opt/trn_rl_repo")

from contextlib import ExitStack

import concourse.bass as bass
import concourse.tile as tile
from concourse import bacc, mybir
from concourse.bass_utils import run_bass_kernel_spmd

B, N, L = 16, 32, 40
NCORES = 8
B_LOC = B // NCORES  # 2
P = 128  # partitions = B_LOC * N * 2
NT = 264           # packed pairs per partition-half
NOFF = 248         # off-diagonal entries (first NOFF of NT); rest are diagonal

BOHR = 0.52917721092
ITA = 1.12
ZETA = 4.0
RS_VALS = np.array([0.5, 1.17, 1.83, 2.5, 3.17]) / BOHR
NR, NM = 5, 4

F32 = mybir.dt.float32
I32 = mybir.dt.int32
OP = mybir.AluOpType
ACT = mybir.ActivationFunctionType

# free-axis offsets in the single packed [128, NIN] input tensor
OFF_CI = 0                 # [3]       coords of atom i (per-partition scalars)
OFF_CJ = 3                 # [3*NT]    coords of j-atom of pair t, layout [c][t]
OFF_CK = 3 + 3 * NT        # [3*NT]    coords of k-atom of pair t
OFF_UJ = 3 + 6 * NT        # [NT]      d[b,i,j_t]
OFF_UK = OFF_UJ + NT       # [NT]      d[b,i,k_t]
OFF_FJ = OFF_UK + NT       # [NT]      d_cutoff[b,i,j_t]
OFF_FK = OFF_FJ + NT       # [NT]      d_cutoff[b,i,k_t]
NIN = 3 + 10 * NT


def _pair_index():
    """Static (j,k) pair enumeration: per half, 248 off-diagonal + 16 diagonal."""
    pairs = [(j, k) for j in range(N) for k in range(j + 1, N)]  # 496
    halves = [pairs[0::2], pairs[1::2]]
    tri_j = np.zeros((2, NT), dtype=np.int64)
    tri_k = np.zeros((2, NT), dtype=np.int64)
    for h in range(2):
        for t, (j, k) in enumerate(halves[h]):
            tri_j[h, t], tri_k[h, t] = j, k
        for t2, j in enumerate(range(h * 16, (h + 1) * 16)):
            tri_j[h, NOFF + t2] = tri_k[h, NOFF + t2] = j
    return tri_j, tri_k


_TRI_J, _TRI_K = _pair_index()


def _build():
    nc = bacc.Bacc("TRN2", target_bir_lowering=False, debug=False)
    inp_d = nc.declare_dram_parameter("inp", [P, NIN], F32, isOutput=False)
    cst_d = nc.declare_dram_parameter("cst", [P, 64], F32, isOutput=False)
    out_d = nc.declare_dram_parameter("out", [B_LOC * N, L], F32, isOutput=True)

    TWO_PI = float(2.0 * np.pi)

    with tile.TileContext(nc) as tc, ExitStack() as ctx:
        pool = ctx.enter_context(tc.tile_pool(name="sb", bufs=1))
        rad_pool = ctx.enter_context(tc.tile_pool(name="rad", bufs=2))
        w_pool = ctx.enter_context(tc.tile_pool(name="w", bufs=2))
        scr_pool = ctx.enter_context(tc.tile_pool(name="scr", bufs=3))
        psum = ctx.enter_context(tc.tile_pool(name="ps", bufs=1, space="PSUM"))

        def big(tag, dt=F32):
            return pool.tile([P, NT], dt, name=tag, tag=tag)

        # two large contiguous input DMAs (big descriptors = full DMA bandwidth)
        NCO = 3 + 6 * NT
        geom = pool.tile([P, NCO], F32, name="geom", tag="geom")
        uf = pool.tile([P, 4 * NT], F32, name="uf", tag="uf")
        cst = pool.tile([P, 64], F32, name="cst", tag="cst")
        nc.sync.dma_start(geom[:], inp_d[:, 0:NCO])
        nc.gpsimd.dma_start(uf[:], inp_d[:, NCO : NCO + 4 * NT])
        nc.gpsimd.dma_start(cst[:], cst_d[:])
        ci = geom[:, OFF_CI : OFF_CI + 3]
        cj = geom[:, OFF_CJ : OFF_CJ + 3 * NT].rearrange("p (c t) -> p c t", c=3)
        ck = geom[:, OFF_CK : OFF_CK + 3 * NT].rearrange("p (c t) -> p c t", c=3)
        uj = uf[:, 0 * NT : 1 * NT]
        uk = uf[:, 1 * NT : 2 * NT]
        fj = uf[:, 2 * NT : 3 * NT]
        fk = uf[:, 3 * NT : 4 * NT]

        # ---- vj = xj - xi (= -v_j), vk = xk - xi (= -v_k); dot = vj.vk ----
        vj = pool.tile([P, 3, NT], F32, name="vj", tag="vj")
        vk = pool.tile([P, 3, NT], F32, name="vk", tag="vk")
        for c in range(3):
            nc.vector.tensor_scalar(vj[:, c, :], cj[:, c, :], ci[:, c : c + 1], None, OP.subtract)
            nc.vector.tensor_scalar(vk[:, c, :], ck[:, c, :], ci[:, c : c + 1], None, OP.subtract)

        prod = big("prod")
        dot = big("dot")
        nc.vector.tensor_tensor(dot[:], vj[:, 0, :], vk[:, 0, :], OP.mult)
        for c in (1, 2):
            nc.vector.tensor_tensor(prod[:], vj[:, c, :], vk[:, c, :], OP.mult)
            nc.vector.tensor_tensor(dot[:], dot[:], prod[:], OP.add)

        # ---- thp = theta / 2pi  (theta = dot / (uj*uk + 1e-5)) ----
        den = big("den")
        nc.gpsimd.tensor_tensor(den[:], uj, uk, OP.mult)
        nc.vector.tensor_scalar(den[:], den[:], 1e-5, TWO_PI, OP.add, OP.mult)
        rden = big("rden")
        nc.vector.reciprocal_approx_fast(rden[:], den[:])
        thp = big("thp")
        nc.vector.tensor_tensor(thp[:], dot[:], rden[:], OP.mult)

        # ---- radial stage (emitted before trig so ACT groups Exp with Square:
        #      exp_and_friends loads once, then trig_and_small once) ----
        q = big("q")
        nc.gpsimd.tensor_tensor(q[:], uj, uk, OP.add)
        cut = big("cut")
        nc.vector.scalar_tensor_tensor(
            cut[:, :NOFF], fj[:, :NOFF], 0.25, fk[:, :NOFF], OP.mult, OP.mult)
        nc.vector.scalar_tensor_tensor(
            cut[:, NOFF:], fj[:, NOFF:], 0.125, fk[:, NOFF:], OP.mult, OP.mult)
        rs_bias = pool.tile([P, NR], F32, name="rs_bias", tag="rs_bias")
        for r in range(NR):
            nc.vector.memset(rs_bias[:, r : r + 1], float(-RS_VALS[r]))
        Ws = []
        exp_insts = []
        for r in range(NR):
            sq = rad_pool.tile([P, NT], F32, name=f"sq{r}", tag="sq")
            nc.scalar.activation(sq[:], q[:], ACT.Square, bias=rs_bias[:, r : r + 1], scale=0.5)
            rad = rad_pool.tile([P, NT], F32, name=f"rad{r}", tag="rad")
            exp_insts.append(nc.scalar.activation(rad[:], sq[:], ACT.Exp, scale=float(-ITA)))
            W = w_pool.tile([P, NT], F32, name=f"w{r}", tag=f"w{r}")
            nc.gpsimd.tensor_tensor(W[:], cut[:], rad[:], OP.mult)
            Ws.append(W)

        # ---- range-reduce + sin/cos via Sin table ----
        # round(r) for |r| < 2^22 via the f32 magic constant: (r + 1.5*2^23) - 1.5*2^23
        RC = float(12582912.0)

        def trig(src, out_tag):
            n_f = big(out_tag + "_nf")
            nc.vector.tensor_scalar(n_f[:], src[:], RC, RC, OP.add, OP.subtract)
            fr = big(out_tag + "_fr")
            nc.gpsimd.tensor_tensor(fr[:], src[:], n_f[:], OP.subtract)
            o = big(out_tag)
            sin_inst = nc.scalar.activation(o[:], fr[:], ACT.Sin, scale=TWO_PI)
            return o, sin_inst

        s1, sin1_inst = trig(thp, "s1")
        r2 = big("r2")
        nc.vector.tensor_scalar(r2[:], thp[:], 0.25, None, OP.add)
        c1, sin2_inst = trig(r2, "c1")

        from concourse.tile import add_dep_helper
        for ei in exp_insts:
            add_dep_helper(sin1_inst.ins, ei.ins, sync=False, reason="group exp-set before trig-set")
            add_dep_helper(sin2_inst.ins, ei.ins, sync=False, reason="group exp-set before trig-set")

        # ---- 4 angular fields (1+-c)^4, (1+-s)^4 via two chained Squares on ScalarE ----
        bias_one = pool.tile([P, 1], F32, name="bias_one", tag="bias_one")
        nc.vector.memset(bias_one[:], 1.0)
        angs = []
        for nm, src, sc in (("bp", s1, 1.0), ("bm", s1, -1.0), ("ap", c1, 1.0), ("am", c1, -1.0)):
            g = big("g_" + nm)
            nc.scalar.activation(g[:], src[:], ACT.Square, bias=bias_one[:], scale=sc)
            a = big("ang_" + nm)
            nc.scalar.activation(a[:], g[:], ACT.Square)
            angs.append(a)


        spart = pool.tile([P, 24], F32, name="spart", tag="spart")

        # ---- fused reduces sum_t W_r * ang_m ; angs order (bp,bm,ap,am) -> m col (1,3,0,2)
        for mi, mcol in ((0, 1), (1, 3), (2, 0), (3, 2)):
            for r in range(NR):
                scr = scr_pool.tile([P, NT], F32, name=f"scr{r}{mcol}", tag="scr")
                nc.vector.scalar_tensor_tensor(
                    scr[:], Ws[r][:], 0.0, angs[mi][:], OP.bypass, OP.mult,
                    accum_out=spart[:, r * NM + mcol : r * NM + mcol + 1])

        # ---- combine: pair-sum over half partitions; assemble all 40 l-columns in PSUM
        #      l = lam*20 + r*4 + t ; lam=+1 -> m=t ; lam=-1 -> m=(t+2)%4 ----
        s2p = psum.tile([64, L], F32, name="s2p", tag="s2p")
        sp3 = spart[:, 0 : NR * NM].rearrange("p (r t) -> p r t", r=NR, t=NM)
        nc.tensor.matmul(s2p[:, 0:20], cst[:, 0:64], spart[:, 0 : NR * NM])
        o3 = s2p[:].rearrange("n (g r t) -> n g r t", g=2, r=NR, t=NM)
        nc.tensor.matmul(o3[:, 1, :, 0:2], cst[:, 0:64], sp3[:, :, 2:4])
        nc.tensor.matmul(o3[:, 1, :, 2:4], cst[:, 0:64], sp3[:, :, 0:2])
        s2s = pool.tile([64, L], F32, name="s2s", tag="s2s")
        nc.vector.tensor_copy(s2s[:], s2p[:])
        nc.sync.dma_start(out_d[:], s2s[:])

    nc.compile()
    return nc


def _ensure_ntff_hook():
    """Register the axon NTFF profiling hook if the image lacks antenv.axon_hooks."""
    import types

    try:
        from antenv.axon_hooks import get_axon_ntff_profile_hook
        if get_axon_ntff_profile_hook() is not None:
            return
        have_mod = True
    except ImportError:
        have_mod = False
    try:
        if "/root/.axon_site" not in sys.path:
            sys.path.insert(0, "/root/.axon_site")
        from trn_agent_boot.trn_boot import _ntff_profile_via_ctypes

        hook = _ntff_profile_via_ctypes("/opt/axon/libaxon_pjrt.so")
        if hook is None:
            return
    except Exception:
        return
    if have_mod:
        from antenv import axon_hooks
        axon_hooks.set_axon_ntff_profile_hook(hook)
    else:
        m = types.ModuleType("antenv.axon_hooks")
        _h = [hook]
        m.get_axon_ntff_profile_hook = lambda: _h[0]
        m.set_axon_ntff_profile_hook = lambda h: _h.__setitem__(0, h)
        import antenv
        antenv.axon_hooks = m
        sys.modules["antenv.axon_hooks"] = m


_NC = None


def _get_nc():
    global _NC
    if _NC is None:
        _NC = _build()
    return _NC


def _host_pack(d_cutoff, d, atom_coordinates):
    """Pure gather/replication of raw inputs into the per-core packed layout."""
    d_cutoff = np.ascontiguousarray(d_cutoff, dtype=np.float32)
    d = np.ascontiguousarray(d, dtype=np.float32)
    coords = np.ascontiguousarray(atom_coordinates, dtype=np.float32)

    p = np.arange(P)
    b_of_p = p // (N * 2)          # [P]
    i_of_p = (p // 2) % N          # [P]
    half = p % 2                   # [P]
    jt = _TRI_J[half]              # [P, NT]
    kt = _TRI_K[half]              # [P, NT]

    in_maps = []
    for c in range(NCORES):
        cd = coords[c * B_LOC : (c + 1) * B_LOC]
        dd = d[c * B_LOC : (c + 1) * B_LOC]
        fc = d_cutoff[c * B_LOC : (c + 1) * B_LOC]
        buf = np.empty((P, NIN), dtype=np.float32)
        buf[:, OFF_CI : OFF_CI + 3] = cd[b_of_p, i_of_p]
        buf[:, OFF_CJ : OFF_CJ + 3 * NT] = (
            cd[b_of_p[:, None], jt].transpose(0, 2, 1).reshape(P, 3 * NT))
        buf[:, OFF_CK : OFF_CK + 3 * NT] = (
            cd[b_of_p[:, None], kt].transpose(0, 2, 1).reshape(P, 3 * NT))
        buf[:, OFF_UJ : OFF_UJ + NT] = dd[b_of_p[:, None], i_of_p[:, None], jt]
        buf[:, OFF_UK : OFF_UK + NT] = dd[b_of_p[:, None], i_of_p[:, None], kt]
        buf[:, OFF_FJ : OFF_FJ + NT] = fc[b_of_p[:, None], i_of_p[:, None], jt]
        buf[:, OFF_FK : OFF_FK + NT] = fc[b_of_p[:, None], i_of_p[:, None], kt]
        in_maps.append({"inp": buf, "cst": _const_blob()})
    return in_maps


_CST = None


def _const_blob():
    global _CST
    if _CST is None:
        cst = np.zeros((P, 64), dtype=np.float32)
        cst[:, 0:64] = np.repeat(np.eye(64, dtype=np.float32), 2, axis=0)
        _CST = cst
    return _CST


def kernel(d_cutoff, d, atom_coordinates, _trace=False):
    if _trace:
        _ensure_ntff_hook()
    nc = _get_nc()
    in_maps = _host_pack(d_cutoff, d, atom_coordinates)
    res = run_bass_kernel_spmd(nc, in_maps, core_ids=list(range(NCORES)), trace=_trace)
    out = np.concatenate(
        [res.results[c]["out"].reshape(B_LOC, N, L) for c in range(NCORES)], axis=0
    ).astype(np.float32)
    if _trace:
        kernel._last_results = res
    return out


# revision 19
# speedup vs baseline: 1.0517x; 1.0517x over previous
"""Trainium2 Bass kernel for AngularSymmetryMod (ANI-style angular symmetry functions).

Math: out[b,i,l] = sum_{j,k} (1+lam*cos(theta-theta_t))^zeta * exp(-ita*((R_ij+R_ik)/2-Rs)^2)
                            * f_ij*f_ik * 2^(1-zeta)
over a 40-point parameter grid l=(lam in {+-1}, 5 Rs values, 4 theta_t values), zeta=4.

Key optimizations:
 1. theta_t = {0.0, 1.57, 3.14, 4.71} are (to 8e-4) the exact quadrants {0, pi/2, pi, 3pi/2},
    so cos(theta-theta_t) = {c, s, -c, -s} and the angular factor collapses to FOUR distinct
    fields: (1+-c)^4, (1+-s)^4 — each two chained Square activations on the ScalarEngine.
    (Validated: 2.2e-4 rel err vs the f32 reference.)
 2. Each of the 40 outputs is S[r, m] (5 radials x 4 angulars = 20 reductions); the 40 outputs
    are a column remap handled by the output DMA.
 3. The (j,k) summand is symmetric, so only the 528 pairs j<=k are computed (host gathers the
    packed pair layout; off-diagonal weight 2 is folded into the cutoff product on-chip).
 4. sin/cos need exact-range reduction (theta spans +-2.3e6): theta/2pi - round(theta/2pi) via
    the DVE's round-to-nearest f32->int32 convert, then the Sin table on [-pi, pi].

Sharding: data-parallel over batch (16 molecules -> 2 per core on 8 cores). No collectives.
Layout per core: 128 partitions = (b_loc:2, i:32, half:2), free = 264 packed (j,k) pairs
(248 off-diagonal + 16 diagonal per half).
"""

import sys
import numpy as np

sys.path.insert(0, "/opt/trn_rl_repo")

from contextlib import ExitStack

import concourse.bass as bass
import concourse.tile as tile
from concourse import bacc, mybir
from concourse.bass_utils import run_bass_kernel_spmd

B, N, L = 16, 32, 40
NCORES = 8
B_LOC = B // NCORES  # 2
P = 128  # partitions = B_LOC * N * 2
NT = 264           # packed pairs per partition-half
NOFF = 248         # off-diagonal entries (first NOFF of NT); rest are diagonal

BOHR = 0.52917721092
ITA = 1.12
ZETA = 4.0
RS_VALS = np.array([0.5, 1.17, 1.83, 2.5, 3.17]) / BOHR
NR, NM = 5, 4

F32 = mybir.dt.float32
I32 = mybir.dt.int32
OP = mybir.AluOpType
ACT = mybir.ActivationFunctionType

# free-axis offsets: [ci(3) | per-coordinate (cj_c, ck_c) pair blocks | u/f block]
OFF_CI = 0                  # [3]    coords of atom i (per-partition scalars)
OFF_C0 = 3                  # [2*NT] (cj_x, ck_x) — then y, z blocks of the same shape
OFF_UJ = 3 + 6 * NT         # [NT]   d[b,i,j_t]
OFF_UK = OFF_UJ + NT        # [NT]   d[b,i,k_t]
OFF_FJ = OFF_UK + NT        # [NT]   d_cutoff[b,i,j_t]
OFF_FK = OFF_FJ + NT        # [NT]   d_cutoff[b,i,k_t]
NIN = 3 + 10 * NT


def _pair_index():
    """Static (j,k) pair enumeration: per half, 248 off-diagonal + 16 diagonal."""
    pairs = [(j, k) for j in range(N) for k in range(j + 1, N)]  # 496
    halves = [pairs[0::2], pairs[1::2]]
    tri_j = np.zeros((2, NT), dtype=np.int64)
    tri_k = np.zeros((2, NT), dtype=np.int64)
    for h in range(2):
        for t, (j, k) in enumerate(halves[h]):
            tri_j[h, t], tri_k[h, t] = j, k
        for t2, j in enumerate(range(h * 16, (h + 1) * 16)):
            tri_j[h, NOFF + t2] = tri_k[h, NOFF + t2] = j
    return tri_j, tri_k


_TRI_J, _TRI_K = _pair_index()


def _build():
    nc = bacc.Bacc("TRN2", target_bir_lowering=False, debug=False)
    inp_d = nc.declare_dram_parameter("inp", [P, NIN], F32, isOutput=False)
    cst_d = nc.declare_dram_parameter("cst", [P, 64], F32, isOutput=False)
    out_d = nc.declare_dram_parameter("out", [B_LOC * N, L], F32, isOutput=True)

    TWO_PI = float(2.0 * np.pi)

    with tile.TileContext(nc) as tc, ExitStack() as ctx:
        pool = ctx.enter_context(tc.tile_pool(name="sb", bufs=1))
        rad_pool = ctx.enter_context(tc.tile_pool(name="rad", bufs=2))
        w_pool = ctx.enter_context(tc.tile_pool(name="w", bufs=2))
        scr_pool = ctx.enter_context(tc.tile_pool(name="scr", bufs=3))
        psum = ctx.enter_context(tc.tile_pool(name="ps", bufs=1, space="PSUM"))

        def big(tag, dt=F32):
            return pool.tile([P, NT], dt, name=tag, tag=tag)

        # chunked input DMAs: each coordinate's (cj_c, ck_c) pair block lands separately
        # so the dot-product chain pipelines behind the transfers.
        cic = pool.tile([P, 3], F32, name="cic", tag="cic")
        geo = [pool.tile([P, 2 * NT], F32, name=f"geo{c}", tag=f"geo{c}") for c in range(3)]
        uf = pool.tile([P, 4 * NT], F32, name="uf", tag="uf")
        cst = pool.tile([P, 64], F32, name="cst", tag="cst")
        nc.sync.dma_start(cic[:], inp_d[:, OFF_CI : OFF_CI + 3])
        for c in range(3):
            nc.sync.dma_start(geo[c][:], inp_d[:, OFF_C0 + 2 * NT * c : OFF_C0 + 2 * NT * (c + 1)])
        nc.gpsimd.dma_start(uf[:], inp_d[:, OFF_UJ : OFF_UJ + 4 * NT])
        nc.gpsimd.dma_start(cst[:], cst_d[:])
        uj = uf[:, 0 * NT : 1 * NT]
        uk = uf[:, 1 * NT : 2 * NT]
        fj = uf[:, 2 * NT : 3 * NT]
        fk = uf[:, 3 * NT : 4 * NT]

        # ---- vj = xj - xi (= -v_j), vk = xk - xi (= -v_k); dot = sum_c vj_c*vk_c ----
        vj = pool.tile([P, 3, NT], F32, name="vj", tag="vj")
        vk = pool.tile([P, 3, NT], F32, name="vk", tag="vk")
        prod = big("prod")
        dot = big("dot")
        for c in range(3):
            nc.vector.tensor_scalar(vj[:, c, :], geo[c][:, 0:NT], cic[:, c : c + 1], None, OP.subtract)
            nc.vector.tensor_scalar(vk[:, c, :], geo[c][:, NT : 2 * NT], cic[:, c : c + 1], None, OP.subtract)
            if c == 0:
                nc.vector.tensor_tensor(dot[:], vj[:, 0, :], vk[:, 0, :], OP.mult)
            else:
                nc.vector.tensor_tensor(prod[:], vj[:, c, :], vk[:, c, :], OP.mult)
                nc.vector.tensor_tensor(dot[:], dot[:], prod[:], OP.add)

        # ---- thp = theta / 2pi  (theta = dot / (uj*uk + 1e-5)) ----
        den = big("den")
        nc.gpsimd.tensor_tensor(den[:], uj, uk, OP.mult)
        nc.vector.tensor_scalar(den[:], den[:], 1e-5, TWO_PI, OP.add, OP.mult)
        rden = big("rden")
        nc.vector.reciprocal_approx_fast(rden[:], den[:])
        thp = big("thp")
        nc.vector.tensor_tensor(thp[:], dot[:], rden[:], OP.mult)

        # ---- radial stage (emitted before trig so ACT groups Exp with Square:
        #      exp_and_friends loads once, then trig_and_small once) ----
        q = big("q")
        nc.gpsimd.tensor_tensor(q[:], uj, uk, OP.add)
        cut = big("cut")
        nc.vector.scalar_tensor_tensor(
            cut[:, :NOFF], fj[:, :NOFF], 0.25, fk[:, :NOFF], OP.mult, OP.mult)
        nc.vector.scalar_tensor_tensor(
            cut[:, NOFF:], fj[:, NOFF:], 0.125, fk[:, NOFF:], OP.mult, OP.mult)
        rs_bias = pool.tile([P, NR], F32, name="rs_bias", tag="rs_bias")
        for r in range(NR):
            nc.vector.memset(rs_bias[:, r : r + 1], float(-RS_VALS[r]))
        Ws = []
        exp_insts = []
        for r in range(NR):
            sq = rad_pool.tile([P, NT], F32, name=f"sq{r}", tag="sq")
            nc.scalar.activation(sq[:], q[:], ACT.Square, bias=rs_bias[:, r : r + 1], scale=0.5)
            rad = rad_pool.tile([P, NT], F32, name=f"rad{r}", tag="rad")
            exp_insts.append(nc.scalar.activation(rad[:], sq[:], ACT.Exp, scale=float(-ITA)))
            W = w_pool.tile([P, NT], F32, name=f"w{r}", tag=f"w{r}")
            nc.gpsimd.tensor_tensor(W[:], cut[:], rad[:], OP.mult)
            Ws.append(W)

        # ---- range-reduce + sin/cos via Sin table ----
        # round(r) for |r| < 2^22 via the f32 magic constant: (r + 1.5*2^23) - 1.5*2^23
        RC = float(12582912.0)

        def trig(src, out_tag):
            n_f = big(out_tag + "_nf")
            nc.vector.tensor_scalar(n_f[:], src[:], RC, RC, OP.add, OP.subtract)
            fr = big(out_tag + "_fr")
            nc.vector.tensor_tensor(fr[:], src[:], n_f[:], OP.subtract)
            o = big(out_tag)
            sin_inst = nc.scalar.activation(o[:], fr[:], ACT.Sin, scale=TWO_PI)
            return o, sin_inst

        s1, sin1_inst = trig(thp, "s1")
        r2 = big("r2")
        nc.vector.tensor_scalar(r2[:], thp[:], 0.25, None, OP.add)
        c1, sin2_inst = trig(r2, "c1")

        from concourse.tile import add_dep_helper
        for ei in exp_insts:
            add_dep_helper(sin1_inst.ins, ei.ins, sync=False, reason="group exp-set before trig-set")
            add_dep_helper(sin2_inst.ins, ei.ins, sync=False, reason="group exp-set before trig-set")

        # ---- 4 angular fields (1+-c)^4, (1+-s)^4 via two chained Squares on ScalarE ----
        bias_one = pool.tile([P, 1], F32, name="bias_one", tag="bias_one")
        nc.vector.memset(bias_one[:], 1.0)
        angs = []
        for nm, src, sc in (("bp", s1, 1.0), ("bm", s1, -1.0), ("ap", c1, 1.0), ("am", c1, -1.0)):
            g = big("g_" + nm)
            nc.scalar.activation(g[:], src[:], ACT.Square, bias=bias_one[:], scale=sc)
            a = big("ang_" + nm)
            nc.scalar.activation(a[:], g[:], ACT.Square)
            angs.append(a)


        spart = pool.tile([P, 24], F32, name="spart", tag="spart")

        # ---- fused reduces sum_t W_r * ang_m ; angs order (bp,bm,ap,am) -> m col (1,3,0,2)
        for mi, mcol in ((0, 1), (1, 3), (2, 0), (3, 2)):
            for r in range(NR):
                scr = scr_pool.tile([P, NT], F32, name=f"scr{r}{mcol}", tag="scr")
                nc.vector.scalar_tensor_tensor(
                    scr[:], Ws[r][:], 0.0, angs[mi][:], OP.bypass, OP.mult,
                    accum_out=spart[:, r * NM + mcol : r * NM + mcol + 1])

        # ---- combine: pair-sum over half partitions; assemble all 40 l-columns in PSUM
        #      l = lam*20 + r*4 + t ; lam=+1 -> m=t ; lam=-1 -> m=(t+2)%4 ----
        s2p = psum.tile([64, L], F32, name="s2p", tag="s2p")
        sp3 = spart[:, 0 : NR * NM].rearrange("p (r t) -> p r t", r=NR, t=NM)
        nc.tensor.matmul(s2p[:, 0:20], cst[:, 0:64], spart[:, 0 : NR * NM])
        o3 = s2p[:].rearrange("n (g r t) -> n g r t", g=2, r=NR, t=NM)
        nc.tensor.matmul(o3[:, 1, :, 0:2], cst[:, 0:64], sp3[:, :, 2:4])
        nc.tensor.matmul(o3[:, 1, :, 2:4], cst[:, 0:64], sp3[:, :, 0:2])
        s2s = pool.tile([64, L], F32, name="s2s", tag="s2s")
        nc.vector.tensor_copy(s2s[:], s2p[:])
        nc.sync.dma_start(out_d[:], s2s[:])

    nc.compile()
    return nc


def _ensure_ntff_hook():
    """Register the axon NTFF profiling hook if the image lacks antenv.axon_hooks."""
    import types

    try:
        from antenv.axon_hooks import get_axon_ntff_profile_hook
        if get_axon_ntff_profile_hook() is not None:
            return
        have_mod = True
    except ImportError:
        have_mod = False
    try:
        if "/root/.axon_site" not in sys.path:
            sys.path.insert(0, "/root/.axon_site")
        from trn_agent_boot.trn_boot import _ntff_profile_via_ctypes

        hook = _ntff_profile_via_ctypes("/opt/axon/libaxon_pjrt.so")
        if hook is None:
            return
    except Exception:
        return
    if have_mod:
        from antenv import axon_hooks
        axon_hooks.set_axon_ntff_profile_hook(hook)
    else:
        m = types.ModuleType("antenv.axon_hooks")
        _h = [hook]
        m.get_axon_ntff_profile_hook = lambda: _h[0]
        m.set_axon_ntff_profile_hook = lambda h: _h.__setitem__(0, h)
        import antenv
        antenv.axon_hooks = m
        sys.modules["antenv.axon_hooks"] = m


_NC = None


def _get_nc():
    global _NC
    if _NC is None:
        _NC = _build()
    return _NC


def _host_pack(d_cutoff, d, atom_coordinates):
    """Pure gather/replication of raw inputs into the per-core packed layout."""
    d_cutoff = np.ascontiguousarray(d_cutoff, dtype=np.float32)
    d = np.ascontiguousarray(d, dtype=np.float32)
    coords = np.ascontiguousarray(atom_coordinates, dtype=np.float32)

    p = np.arange(P)
    b_of_p = p // (N * 2)          # [P]
    i_of_p = (p // 2) % N          # [P]
    half = p % 2                   # [P]
    jt = _TRI_J[half]              # [P, NT]
    kt = _TRI_K[half]              # [P, NT]

    in_maps = []
    for c in range(NCORES):
        cd = coords[c * B_LOC : (c + 1) * B_LOC]
        dd = d[c * B_LOC : (c + 1) * B_LOC]
        fc = d_cutoff[c * B_LOC : (c + 1) * B_LOC]
        buf = np.empty((P, NIN), dtype=np.float32)
        buf[:, OFF_CI : OFF_CI + 3] = cd[b_of_p, i_of_p]
        cjv = cd[b_of_p[:, None], jt]   # [P, NT, 3]
        ckv = cd[b_of_p[:, None], kt]   # [P, NT, 3]
        for c in range(3):
            buf[:, OFF_C0 + 2 * NT * c : OFF_C0 + 2 * NT * c + NT] = cjv[:, :, c]
            buf[:, OFF_C0 + 2 * NT * c + NT : OFF_C0 + 2 * NT * (c + 1)] = ckv[:, :, c]
        buf[:, OFF_UJ : OFF_UJ + NT] = dd[b_of_p[:, None], i_of_p[:, None], jt]
        buf[:, OFF_UK : OFF_UK + NT] = dd[b_of_p[:, None], i_of_p[:, None], kt]
        buf[:, OFF_FJ : OFF_FJ + NT] = fc[b_of_p[:, None], i_of_p[:, None], jt]
        buf[:, OFF_FK : OFF_FK + NT] = fc[b_of_p[:, None], i_of_p[:, None], kt]
        in_maps.append({"inp": buf, "cst": _const_blob()})
    return in_maps


_CST = None


def _const_blob():
    global _CST
    if _CST is None:
        cst = np.zeros((P, 64), dtype=np.float32)
        cst[:, 0:64] = np.repeat(np.eye(64, dtype=np.float32), 2, axis=0)
        _CST = cst
    return _CST


def kernel(d_cutoff, d, atom_coordinates, _trace=False):
    if _trace:
        _ensure_ntff_hook()
    nc = _get_nc()
    in_maps = _host_pack(d_cutoff, d, atom_coordinates)
    res = run_bass_kernel_spmd(nc, in_maps, core_ids=list(range(NCORES)), trace=_trace)
    out = np.concatenate(
        [res.results[c]["out"].reshape(B_LOC, N, L) for c in range(NCORES)], axis=0
    ).astype(np.float32)
    if _trace:
        kernel._last_results = res
    return out
